# revision 11
# baseline (speedup 1.0000x reference)
"""GPT2 paged-attention decode kernel for Trainium2 (Bass/Tile), 8-core SPMD.

Problem: B=32 batches, Q=1 query, D=1024, H=16 heads, DH=64, KV cache 8192.
  qkv = hidden @ w_attn + b_attn; split into q, k_cur, v_cur
  attention over concat(cache, current) per (b, h)  [no scaling, no mask]
  out = (softmax contexts) @ w_proj + b_proj

Sharding (v3): head-parallel tensor parallel — core c owns heads {2c, 2c+1}
for ALL 32 batches.  Per-core traffic: K/V shard 256MB (same as batch-DP)
but only a 1.5MB w_attn column slice and a 0.5MB w_proj row slice instead
of the full 16MB of weights.  Each core emits its partial c_proj output
(its heads' contribution); the host sums the 8 partials and adds b_proj —
the TP all-reduce folded into unsharding.

Precision: K/V/q are cast fp32->fp16 during the SWDGE DMA (HBM traffic
unchanged; SBUF footprint and DVE work halved; TensorE matmuls run native
16-bit instead of 2-pass fp32).  e = exp(s) is bf16 (fp16 would overflow:
|s| can reach ~25).  Softmax max-subtraction is dropped: softmax(s) =
exp(s)/sum(exp(s)) exactly, and fp32/bf16 exp cannot overflow here.

Schedule notes:
  - The gpsimd (SWDGE) queue carries ONLY the 128 big K/V cast-DMAs, so the
    256MB stream starts immediately and never stalls behind dependent work:
    the qkv bias is folded into the projection matmul (host appends b_qkv
    as an extra row of w_qkv against a ones-row of hiddenT), qkv is written
    back to DRAM as fp16, and the q-broadcast + current-token loads are
    cast-free fp16 reads on the HWDGE (scalar) queue.
  - Per (b,h) pair: K/V as [128, 65, 64] fp16 tiles (partition p holds keys
    p*64..p*64+63; slot 64 = current token on partition 0, zeroed elsewhere
    so fp16 garbage can't poison the PE accumulate).  VectorE computes
    kq = K*q_bcast and s = reduce_sum(kq) at 16-bit 2x rate; ScalarE does
    e = exp(s) (bf16) with accum_out denominators; TensorE accumulates
    ctx^T via 65 16-bit matmuls (e column stationary, V streaming) plus
    tiny rank-1 matmuls for denominators and the row->column transpose.
  - The epilogue (softmax divide + partial c_proj) runs per head: head 0's
    epilogue executes in the shadow of head 1's KV stream; separate PSUM
    banks per head keep TensorE writes and DVE/ScalarE reads collision-free.
"""

import os
import sys

import numpy as np

sys.path.insert(0, "/opt/trn_rl_repo")

import concourse.bass as bass
import concourse.tile as tile
from concourse import bacc, mybir
from concourse.bass_utils import run_bass_kernel_spmd

FP32 = mybir.dt.float32
BF16 = mybir.dt.bfloat16
FP16 = mybir.dt.float16

# Problem shape (hardcoded per contest rules).
B, D, H, DH, KV = 32, 1024, 16, 64, 8192
NCORES = 8
HC = H // NCORES          # heads per core = 2
NPAIRS = B * HC           # 64 (b,h) pairs per core
QKVC = 3 * HC * DH        # 384 qkv columns per core
DAUG = D + 128            # hidden dim + bias row, zero-padded to 9*128
JT = KV // 128            # 64 key-slots per partition
JT1 = JT + 1              # +1 slot for the current token


def _bcast_ap(t_ap, col0, ncols, nparts, row_stride):
    """DRAM AP [nparts, rows, ncols] with partition stride 0 (broadcast)."""
    return bass.AP(
        tensor=t_ap.tensor,
        offset=t_ap.offset + col0,
        ap=[[0, nparts], [row_stride, t_ap.shape[0]], [1, ncols]],
    )


def build_kernel():
    nc = bacc.Bacc(
        "TRN2",
        target_bir_lowering=False,
        debug=False,
        enable_asserts=False,
        num_devices=NCORES,
    )

    hiddenT = nc.dram_tensor("hiddenT", [128, 9 * B], FP32, kind="ExternalInput")
    k_cache = nc.dram_tensor("k_cache", [B, HC, KV, DH], FP32, kind="ExternalInput")
    v_cache = nc.dram_tensor("v_cache", [B, HC, KV, DH], FP32, kind="ExternalInput")
    w_qkv = nc.dram_tensor("w_qkv", [128, 9 * QKVC], FP32, kind="ExternalInput")
    w_proj_sl = nc.dram_tensor("w_proj_sl", [64, HC * D], FP32, kind="ExternalInput")
    out = nc.dram_tensor("out", [B, D], FP32, kind="ExternalOutput")

    with tile.TileContext(nc) as tc:
        with (
            tc.tile_pool(name="singles", bufs=1) as singles,
            tc.tile_pool(name="kv_pool", bufs=7) as kv_pool,
            tc.tile_pool(name="kq_pool", bufs=2) as kq_pool,
            tc.tile_pool(name="se_pool", bufs=3) as se_pool,
            tc.tile_pool(name="dram_pool", bufs=1, space="DRAM") as dram_pool,
            tc.tile_pool(name="mm_ps", bufs=2, space="PSUM") as mm_ps,
            tc.tile_pool(name="ctx_ps", bufs=2, space="PSUM") as ctx_ps,
            tc.tile_pool(name="ctxt_ps", bufs=2, space="PSUM") as ctxt_ps,
            tc.tile_pool(name="l_ps", bufs=2, space="PSUM") as l_ps,
        ):
            # ---- constants ----
            ones_f = singles.tile([128, 64], FP32)
            nc.vector.memset(ones_f, 1.0)
            ones_b = singles.tile([128, 1], BF16)
            nc.vector.memset(ones_b, 1.0)

            # ---- QKV projection: qkv = hidden_aug @ w_qkv (bias folded) ----
            # hT[p, t, b] = hiddenT[t*128+p, b]
            hT = singles.tile([128, 9, B], FP32)
            nc.sync.dma_start(
                out=hT, in_=hiddenT.ap().rearrange("p (t b) -> p t b", t=9)
            )
            wq_sb = singles.tile([128, 9, QKVC], FP32)
            nc.sync.dma_start(
                out=wq_sb, in_=w_qkv.ap().rearrange("p (t n) -> p t n", t=9)
            )
            # c_proj weights preloaded up front so the epilogue never waits
            wp_sb = singles.tile([64, HC, D], FP32)
            nc.scalar.dma_start(
                out=wp_sb, in_=w_proj_sl.ap().rearrange("p (hh n) -> p hh n", hh=HC)
            )

            ps_qkv = mm_ps.tile([B, QKVC], FP32, tag="mm")
            for t in range(9):
                nc.tensor.matmul(
                    ps_qkv, hT[:, t, :], wq_sb[:, t, :], start=(t == 0), stop=(t == 8)
                )
            qkv16_sb = singles.tile([B, QKVC], FP16)
            nc.scalar.copy(qkv16_sb, ps_qkv)

            # round-trip qkv (fp16) through DRAM to broadcast q across partitions
            qkv_dram = dram_pool.tile([B, QKVC], FP16)
            nc.sync.dma_start(out=qkv_dram, in_=qkv16_sb)
            # qb_all[p, b, :] = q row of batch b (fp16), same for every p
            qb_all = singles.tile([128, B, HC * DH], FP16)
            nc.scalar.dma_start(
                out=qb_all, in_=_bcast_ap(qkv_dram, 0, HC * DH, 128, QKVC)
            )

            # ---- attention over pairs (h-major so c_proj slices are contiguous) ----
            # per-head PSUM accumulators in distinct banks (collision-free overlap)
            psum_ctxT = [
                ctxt_ps.tile([64, B], FP32, tag="ctxT", name=f"ctxT{h}")
                for h in range(HC)
            ]
            psum_l = [
                l_ps.tile([1, B], FP32, tag="l", name=f"l{h}") for h in range(HC)
            ]
            ps_o = [
                mm_ps.tile([B, 512], FP32, tag="mm", name=f"ps_o{i}") for i in range(2)
            ]
            out_sb = singles.tile([B, D], FP32)

            for j in range(NPAIRS):
                h, b = j // B, j % B

                k_sb = kv_pool.tile([128, JT1, DH], FP16, tag="k")
                v_sb = kv_pool.tile([128, JT1, DH], FP16, tag="v")
                # The final pair streams in two halves so its first half's
                # compute overlaps the second half's DMA (shorter tail).
                cuts = [0, 32, JT1] if j == NPAIRS - 1 else [0, JT1]
                halves = list(zip(cuts[:-1], cuts[1:]))
                kc_ap = k_cache.ap()[b, h].rearrange("(p jj) d -> p jj d", jj=JT)
                vc_ap = v_cache.ap()[b, h].rearrange("(p jj) d -> p jj d", jj=JT)
                for lo, hi in halves:
                    hc_ = min(hi, JT)  # slot JT is fed from qkv_dram, not the cache
                    nc.gpsimd.dma_start(out=k_sb[:, lo:hc_, :], in_=kc_ap[:, lo:hc_, :])
                    nc.gpsimd.dma_start(out=v_sb[:, lo:hc_, :], in_=vc_ap[:, lo:hc_, :])
                # current-token slot: row 0 = k_cur/v_cur.  v rows 1.. must be
                # zeroed (fp16 garbage could be NaN; NaN*0 = NaN in the PE);
                # k likewise for the sim's uninitialized-read check.
                nc.vector.memset(k_sb[:, JT, :], 0.0)
                nc.vector.memset(v_sb[:, JT, :], 0.0)
                ck = HC * DH + h * DH      # k_cur col in qkv row [q | k | v]
                cv = 2 * HC * DH + h * DH  # v_cur col
                nc.scalar.dma_start(
                    out=k_sb[0:1, JT, :], in_=qkv_dram[b : b + 1, ck : ck + DH]
                )
                nc.scalar.dma_start(
                    out=v_sb[0:1, JT, :], in_=qkv_dram[b : b + 1, cv : cv + DH]
                )

                # s[p, jj] = sum_d K[p, jj, d] * q[d]; e = exp(s) (bf16) with
                # accum_out giving per-partition denominator partial sums
                kq = kq_pool.tile([128, JT1, DH], FP16, tag="kq")
                qb = qb_all[:, b, h * DH : (h + 1) * DH].unsqueeze(1)
                s_sb = se_pool.tile([128, JT1], FP32, tag="s")
                e_sb = se_pool.tile([128, JT1], BF16, tag="e")
                lps = []
                for i, (lo, hi) in enumerate(halves):
                    n = hi - lo
                    nc.vector.tensor_mul(
                        kq[:, lo:hi, :], k_sb[:, lo:hi, :], qb.broadcast_to([128, n, DH])
                    )
                    nc.vector.reduce_sum(
                        s_sb[:, lo:hi], kq[:, lo:hi, :], axis=mybir.AxisListType.X
                    )
                    he = min(hi, JT)  # e accum covers the cache part only
                    lp = se_pool.tile([128, 1], FP32, tag=f"lp{i}")
                    nc.scalar.activation(
                        e_sb[:, lo:he],
                        s_sb[:, lo:he],
                        mybir.ActivationFunctionType.Exp,
                        accum_out=lp,
                    )
                    lps.append(lp)
                nc.vector.memset(e_sb[:, JT : JT + 1], 0.0)
                nc.scalar.activation(
                    e_sb[0:1, JT : JT + 1],
                    s_sb[0:1, JT : JT + 1],
                    mybir.ActivationFunctionType.Exp,
                )

                # ctx row [1, dh]: e column stationary, V tiles streaming
                psum_row = ctx_ps.tile([1, DH], FP32, tag="ctx")
                for jj in range(JT1):
                    nc.tensor.matmul(
                        psum_row,
                        e_sb[:, jj : jj + 1],
                        v_sb[:, jj, :],
                        start=(jj == 0),
                        stop=(jj == JT),
                    )
                # denominator: l[h][b] = sum_p sum_i lps[i][p] + e_cur
                for i, lp in enumerate(lps):
                    nc.tensor.matmul(
                        psum_l[h][:, b : b + 1],
                        lp,
                        ones_f[:, 0:1],
                        start=(i == 0),
                        stop=False,
                        skip_group_check=True,
                    )
                nc.tensor.matmul(
                    psum_l[h][:, b : b + 1],
                    e_sb[0:1, JT : JT + 1],
                    ones_b[0:1, 0:1],
                    start=False,
                    stop=True,
                    skip_group_check=True,
                )
                # transpose the row into column b of this head's ctx^T
                ctx_row = se_pool.tile([1, DH], FP32, tag="ctxrow")
                nc.scalar.copy(ctx_row, psum_row)
                nc.tensor.matmul(
                    psum_ctxT[h][:, b : b + 1],
                    ctx_row,
                    ones_f[0:1, 0:1],
                    start=True,
                    stop=True,
                    skip_group_check=True,
                )

                # ---- per-head epilogue, in the shadow of the next head's stream
                if b == B - 1:
                    r_sb = singles.tile([1, B], FP32, name=f"r{h}")
                    nc.vector.reciprocal(r_sb, psum_l[h])
                    # broadcast r across 64 partitions: ones[64]^T (x) r
                    psum_rb = ctx_ps.tile([64, B], FP32, tag="ctx", name=f"rb{h}")
                    nc.tensor.matmul(
                        psum_rb, ones_f[0:1, :], r_sb, start=True, stop=True
                    )
                    rb_sb = singles.tile([64, B], FP32, name=f"rbs{h}")
                    nc.scalar.copy(rb_sb, psum_rb)
                    ctx_scaled = singles.tile([64, B], FP32, name=f"cs{h}")
                    nc.vector.tensor_mul(ctx_scaled, psum_ctxT[h], rb_sb)
                    # partial c_proj: out[b,:] += ctx[b,h,:] @ wp[h]
                    for nb in range(2):
                        nc.tensor.matmul(
                            ps_o[nb],
                            ctx_scaled,
                            wp_sb[:, h, nb * 512 : (nb + 1) * 512],
                            start=(h == 0),
                            stop=(h == HC - 1),
                        )
                    if h == HC - 1:
                        for nb in range(2):
                            nc.scalar.copy(
                                out_sb[:, nb * 512 : (nb + 1) * 512], ps_o[nb]
                            )
                        nc.sync.dma_start(out=out.ap(), in_=out_sb)

    nc.compile()
    return nc


_NC_CACHE = None


def _get_nc():
    global _NC_CACHE
    if _NC_CACHE is None:
        _NC_CACHE = build_kernel()
    return _NC_CACHE


def make_in_maps(inputs):
    """Shard full inputs into per-core input maps (head tensor-parallel)."""
    hidden = np.asarray(inputs["hidden_states"], np.float32).reshape(B, D)
    hiddenT = np.zeros((DAUG, B), np.float32)
    hiddenT[:D] = hidden.T
    hiddenT[D] = 1.0  # ones row multiplying the bias row of w_qkv
    # pre-tile to the SBUF layout: hT[p, t, b] = hiddenT[t*128+p, b]
    hT_tiled = np.ascontiguousarray(
        hiddenT.reshape(9, 128, B).transpose(1, 0, 2).reshape(128, 9 * B)
    )
    k_cache = np.asarray(inputs["k_cache"], np.float32)
    v_cache = np.asarray(inputs["v_cache"], np.float32)
    w_attn = np.asarray(inputs["w_attn"], np.float32)
    b_attn = np.asarray(inputs["b_attn"], np.float32)
    w_proj = np.asarray(inputs["w_proj"], np.float32)
    in_maps = []
    for c in range(NCORES):
        hs = slice(c * HC, (c + 1) * HC)           # heads of this core
        cs = slice(c * HC * DH, (c + 1) * HC * DH)  # their q/k/v column block
        w_qkv = np.zeros((DAUG, QKVC), np.float32)
        w_qkv[:D] = np.concatenate(
            [w_attn[:, cs], w_attn[:, D:][:, cs], w_attn[:, 2 * D :][:, cs]], axis=1
        )
        w_qkv[D] = np.concatenate(
            [b_attn[cs], b_attn[D:][cs], b_attn[2 * D :][cs]]
        )
        # pre-tile: wq[p, t, n] = w_qkv[t*128+p, n]
        w_qkv_tiled = np.ascontiguousarray(
            w_qkv.reshape(9, 128, QKVC).transpose(1, 0, 2).reshape(128, 9 * QKVC)
        )
        # pre-tile: wp[p, hh, n] = w_proj[c*128 + hh*64 + p, n]
        wp_tiled = np.ascontiguousarray(
            w_proj[c * HC * DH : (c + 1) * HC * DH]
            .reshape(HC, 64, D)
            .transpose(1, 0, 2)
            .reshape(64, HC * D)
        )
        in_maps.append(
            {
                "hiddenT": hT_tiled,
                "k_cache": np.ascontiguousarray(k_cache[:, hs]),
                "v_cache": np.ascontiguousarray(v_cache[:, hs]),
                "w_qkv": w_qkv_tiled,
                "w_proj_sl": wp_tiled,
            }
        )
    return in_maps


def kernel(**inputs):
    nc = _get_nc()
    in_maps = make_in_maps(inputs)
    res = run_bass_kernel_spmd(nc, in_maps, core_ids=list(range(NCORES)))
    acc = np.zeros((B, D), np.float32)
    for c in range(NCORES):
        acc += np.asarray(res.results[c]["out"])
    acc += np.asarray(inputs["b_proj"], np.float32)[None, :]
    return acc.reshape(B, 1, D)


if __name__ == "__main__":
    build_kernel()
    print("kernel built OK")


# revision 12
# speedup vs baseline: 1.0159x; 1.0159x over previous
"""GPT2 paged-attention decode kernel for Trainium2 (Bass/Tile), 8-core SPMD.

Problem: B=32 batches, Q=1 query, D=1024, H=16 heads, DH=64, KV cache 8192.
  qkv = hidden @ w_attn + b_attn; split into q, k_cur, v_cur
  attention over concat(cache, current) per (b, h)  [no scaling, no mask]
  out = (softmax contexts) @ w_proj + b_proj

Sharding: head-parallel tensor parallel — core c owns heads {2c, 2c+1} for
ALL 32 batches.  Per-core traffic: K/V shard 256MB (same as batch-DP) but
only a 1.5MB w_attn column slice and a 0.5MB w_proj row slice instead of
the full 16MB of weights.  Each core emits its partial c_proj output (its
heads' contribution); the host sums the 8 partials and adds b_proj — the
TP all-reduce folded into unsharding.  The kernel is HBM-bound on the K/V
stream (~670-700us at ~380-400GB/s per core); everything else hides under it.

Precision: K/V/q are cast fp32->fp16 during the SWDGE DMA (HBM traffic
unchanged; SBUF footprint and DVE work halved; TensorE matmuls run native
16-bit instead of 2-pass fp32).  e = exp(s) is bf16 (fp16 would overflow:
s can reach ~25).  Softmax max-subtraction is dropped: softmax(s) =
exp(s)/sum(exp(s)) exactly, and fp32/bf16 exp cannot overflow here.
Full-output rel err vs the fp32 reference: ~1.9e-3.

Schedule notes:
  - The gpsimd (SWDGE) queue carries ONLY the big K/V cast-DMAs, so the
    256MB stream starts immediately and never stalls behind dependent work:
    the qkv bias is folded into the projection matmul (host appends b_qkv
    as an extra row of w_qkv against a ones-row of hiddenT), qkv is written
    back to DRAM as fp16, and the q-broadcast + current-token loads are
    cast-free fp16 reads on the HWDGE (scalar) queue.  Weights arrive
    host-pre-tiled so their DMAs are contiguous per partition (the strided
    rearrange loads ran at ~45GB/s and delayed the first pairs' compute).
  - kv_pool bufs=7 gives the stream ~78us of buffer runway to ride out the
    prologue latency before the compute pipeline starts releasing tiles.
  - Per (b,h) pair: K/V as [128, 65, 64] fp16 tiles (partition p holds keys
    p*64..p*64+63; slot 64 = current token on partition 0, zeroed elsewhere
    so fp16 garbage can't poison the PE accumulate).  VectorE computes
    kq = K*q_bcast and s = reduce_sum(kq) at 16-bit 2x rate; ScalarE does
    e = exp(s) (bf16 out) with accum_out denominators (and the PSUM->SBUF
    copies — putting those on VectorE serializes DVE against TensorE and
    costs ~8%); TensorE accumulates ctx^T via 65 16-bit matmuls (e column
    stationary, V streaming) plus tiny rank-1 matmuls for denominators and
    the row->column transpose.
  - The epilogue (softmax divide + partial c_proj) runs per head: head 0's
    epilogue executes in the shadow of head 1's KV stream; separate PSUM
    banks per head keep TensorE writes and DVE/ScalarE reads collision-free.
    The final pair streams in two halves to shorten the post-stream tail.
"""

import os
import sys

import numpy as np

sys.path.insert(0, "/opt/trn_rl_repo")

import concourse.bass as bass
import concourse.tile as tile
from concourse import bacc, mybir
from concourse.bass_utils import run_bass_kernel_spmd

FP32 = mybir.dt.float32
BF16 = mybir.dt.bfloat16
FP16 = mybir.dt.float16

# Problem shape (hardcoded per contest rules).
B, D, H, DH, KV = 32, 1024, 16, 64, 8192
NCORES = 8
HC = H // NCORES          # heads per core = 2
NPAIRS = B * HC           # 64 (b,h) pairs per core
QKVC = 3 * HC * DH        # 384 qkv columns per core
DAUG = D + 128            # hidden dim + bias row, zero-padded to 9*128
JT = KV // 128            # 64 key-slots per partition
JT1 = JT + 1              # +1 slot for the current token


def _bcast_ap(t_ap, col0, ncols, nparts, row_stride):
    """DRAM AP [nparts, rows, ncols] with partition stride 0 (broadcast)."""
    return bass.AP(
        tensor=t_ap.tensor,
        offset=t_ap.offset + col0,
        ap=[[0, nparts], [row_stride, t_ap.shape[0]], [1, ncols]],
    )


def build_kernel():
    nc = bacc.Bacc(
        "TRN2",
        target_bir_lowering=False,
        debug=False,
        enable_asserts=False,
        num_devices=NCORES,
    )

    hiddenT = nc.dram_tensor("hiddenT", [128, 9 * B], FP32, kind="ExternalInput")
    k_cache = nc.dram_tensor("k_cache", [B, HC, KV, DH], FP32, kind="ExternalInput")
    v_cache = nc.dram_tensor("v_cache", [B, HC, KV, DH], FP32, kind="ExternalInput")
    w_qkv = nc.dram_tensor("w_qkv", [128, 9 * QKVC], FP32, kind="ExternalInput")
    w_proj_sl = nc.dram_tensor("w_proj_sl", [64, HC * D], FP32, kind="ExternalInput")
    out = nc.dram_tensor("out", [B, D], FP32, kind="ExternalOutput")

    with tile.TileContext(nc) as tc:
        with (
            tc.tile_pool(name="singles", bufs=1) as singles,
            tc.tile_pool(name="kv_pool", bufs=7) as kv_pool,
            tc.tile_pool(name="kq_pool", bufs=2) as kq_pool,
            tc.tile_pool(name="se_pool", bufs=3) as se_pool,
            tc.tile_pool(name="dram_pool", bufs=1, space="DRAM") as dram_pool,
            tc.tile_pool(name="mm_ps", bufs=2, space="PSUM") as mm_ps,
            tc.tile_pool(name="ctx_ps", bufs=2, space="PSUM") as ctx_ps,
            tc.tile_pool(name="ctxt_ps", bufs=2, space="PSUM") as ctxt_ps,
            tc.tile_pool(name="l_ps", bufs=2, space="PSUM") as l_ps,
        ):
            # ---- constants ----
            ones_f = singles.tile([128, 64], FP32)
            nc.vector.memset(ones_f, 1.0)
            ones_b = singles.tile([128, 1], BF16)
            nc.vector.memset(ones_b, 1.0)

            # ---- QKV projection: qkv = hidden_aug @ w_qkv (bias folded) ----
            # hT[p, t, b] = hiddenT[t*128+p, b]
            hT = singles.tile([128, 9, B], FP32)
            nc.sync.dma_start(
                out=hT, in_=hiddenT.ap().rearrange("p (t b) -> p t b", t=9)
            )
            wq_sb = singles.tile([128, 9, QKVC], FP32)
            nc.sync.dma_start(
                out=wq_sb, in_=w_qkv.ap().rearrange("p (t n) -> p t n", t=9)
            )
            # c_proj weights preloaded up front so the epilogue never waits
            wp_sb = singles.tile([64, HC, D], FP32)
            nc.scalar.dma_start(
                out=wp_sb, in_=w_proj_sl.ap().rearrange("p (hh n) -> p hh n", hh=HC)
            )

            ps_qkv = mm_ps.tile([B, QKVC], FP32, tag="mm")
            for t in range(9):
                nc.tensor.matmul(
                    ps_qkv, hT[:, t, :], wq_sb[:, t, :], start=(t == 0), stop=(t == 8)
                )
            qkv16_sb = singles.tile([B, QKVC], FP16)
            nc.scalar.copy(qkv16_sb, ps_qkv)

            # round-trip qkv (fp16) through DRAM to broadcast q across partitions
            qkv_dram = dram_pool.tile([B, QKVC], FP16)
            nc.sync.dma_start(out=qkv_dram, in_=qkv16_sb)
            # qb_all[p, b, :] = q row of batch b (fp16), same for every p
            qb_all = singles.tile([128, B, HC * DH], FP16)
            nc.scalar.dma_start(
                out=qb_all, in_=_bcast_ap(qkv_dram, 0, HC * DH, 128, QKVC)
            )

            # ---- attention over pairs (h-major so c_proj slices are contiguous) ----
            # per-head PSUM accumulators in distinct banks (collision-free overlap)
            psum_ctxT = [
                ctxt_ps.tile([64, B], FP32, tag="ctxT", name=f"ctxT{h}")
                for h in range(HC)
            ]
            psum_l = [
                l_ps.tile([1, B], FP32, tag="l", name=f"l{h}") for h in range(HC)
            ]
            ps_o = [
                mm_ps.tile([B, 512], FP32, tag="mm", name=f"ps_o{i}") for i in range(2)
            ]
            out_sb = singles.tile([B, D], FP32)

            for j in range(NPAIRS):
                h, b = j // B, j % B

                k_sb = kv_pool.tile([128, JT1, DH], FP16, tag="k")
                v_sb = kv_pool.tile([128, JT1, DH], FP16, tag="v")
                # The final pair streams in two halves so its first half's
                # compute overlaps the second half's DMA (shorter tail).
                cuts = [0, 32, JT1] if j == NPAIRS - 1 else [0, JT1]
                halves = list(zip(cuts[:-1], cuts[1:]))
                kc_ap = k_cache.ap()[b, h].rearrange("(p jj) d -> p jj d", jj=JT)
                vc_ap = v_cache.ap()[b, h].rearrange("(p jj) d -> p jj d", jj=JT)
                for lo, hi in halves:
                    hc_ = min(hi, JT)  # slot JT is fed from qkv_dram, not the cache
                    nc.gpsimd.dma_start(out=k_sb[:, lo:hc_, :], in_=kc_ap[:, lo:hc_, :])
                    nc.gpsimd.dma_start(out=v_sb[:, lo:hc_, :], in_=vc_ap[:, lo:hc_, :])
                # current-token slot: row 0 = k_cur/v_cur.  v rows 1.. must be
                # zeroed (fp16 garbage could be NaN; NaN*0 = NaN in the PE);
                # k likewise for the sim's uninitialized-read check.
                nc.vector.memset(k_sb[:, JT, :], 0.0)
                nc.vector.memset(v_sb[:, JT, :], 0.0)
                ck = HC * DH + h * DH      # k_cur col in qkv row [q | k | v]
                cv = 2 * HC * DH + h * DH  # v_cur col
                nc.scalar.dma_start(
                    out=k_sb[0:1, JT, :], in_=qkv_dram[b : b + 1, ck : ck + DH]
                )
                nc.scalar.dma_start(
                    out=v_sb[0:1, JT, :], in_=qkv_dram[b : b + 1, cv : cv + DH]
                )

                # s[p, jj] = sum_d K[p, jj, d] * q[d]; e = exp(s) (bf16) with
                # accum_out giving per-partition denominator partial sums
                kq = kq_pool.tile([128, JT1, DH], FP16, tag="kq")
                qb = qb_all[:, b, h * DH : (h + 1) * DH].unsqueeze(1)
                s_sb = se_pool.tile([128, JT1], FP32, tag="s")
                e_sb = se_pool.tile([128, JT1], BF16, tag="e")
                lps = []
                for i, (lo, hi) in enumerate(halves):
                    n = hi - lo
                    nc.vector.tensor_mul(
                        kq[:, lo:hi, :], k_sb[:, lo:hi, :], qb.broadcast_to([128, n, DH])
                    )
                    nc.vector.reduce_sum(
                        s_sb[:, lo:hi], kq[:, lo:hi, :], axis=mybir.AxisListType.X
                    )
                    he = min(hi, JT)  # e accum covers the cache part only
                    lp = se_pool.tile([128, 1], FP32, tag=f"lp{i}")
                    nc.scalar.activation(
                        e_sb[:, lo:he],
                        s_sb[:, lo:he],
                        mybir.ActivationFunctionType.Exp,
                        accum_out=lp,
                    )
                    lps.append(lp)
                nc.vector.memset(e_sb[:, JT : JT + 1], 0.0)
                nc.scalar.activation(
                    e_sb[0:1, JT : JT + 1],
                    s_sb[0:1, JT : JT + 1],
                    mybir.ActivationFunctionType.Exp,
                )

                # ctx row [1, dh]: e column stationary, V tiles streaming
                psum_row = ctx_ps.tile([1, DH], FP32, tag="ctx")
                for jj in range(JT1):
                    nc.tensor.matmul(
                        psum_row,
                        e_sb[:, jj : jj + 1],
                        v_sb[:, jj, :],
                        start=(jj == 0),
                        stop=(jj == JT),
                    )
                # denominator: l[h][b] = sum_p sum_i lps[i][p] + e_cur
                for i, lp in enumerate(lps):
                    nc.tensor.matmul(
                        psum_l[h][:, b : b + 1],
                        lp,
                        ones_f[:, 0:1],
                        start=(i == 0),
                        stop=False,
                        skip_group_check=True,
                    )
                nc.tensor.matmul(
                    psum_l[h][:, b : b + 1],
                    e_sb[0:1, JT : JT + 1],
                    ones_b[0:1, 0:1],
                    start=False,
                    stop=True,
                    skip_group_check=True,
                )
                # transpose the row into column b of this head's ctx^T
                ctx_row = se_pool.tile([1, DH], FP32, tag="ctxrow")
                nc.scalar.copy(ctx_row, psum_row)
                nc.tensor.matmul(
                    psum_ctxT[h][:, b : b + 1],
                    ctx_row,
                    ones_f[0:1, 0:1],
                    start=True,
                    stop=True,
                    skip_group_check=True,
                )

                # ---- per-head epilogue, in the shadow of the next head's stream
                if b == B - 1:
                    r_sb = singles.tile([1, B], FP32, name=f"r{h}")
                    nc.vector.reciprocal(r_sb, psum_l[h])
                    # broadcast r across 64 partitions: ones[64]^T (x) r
                    psum_rb = ctx_ps.tile([64, B], FP32, tag="ctx", name=f"rb{h}")
                    nc.tensor.matmul(
                        psum_rb, ones_f[0:1, :], r_sb, start=True, stop=True
                    )
                    rb_sb = singles.tile([64, B], FP32, name=f"rbs{h}")
                    nc.scalar.copy(rb_sb, psum_rb)
                    ctx_scaled = singles.tile([64, B], FP32, name=f"cs{h}")
                    nc.vector.tensor_mul(ctx_scaled, psum_ctxT[h], rb_sb)
                    # partial c_proj: out[b,:] += ctx[b,h,:] @ wp[h]
                    for nb in range(2):
                        nc.tensor.matmul(
                            ps_o[nb],
                            ctx_scaled,
                            wp_sb[:, h, nb * 512 : (nb + 1) * 512],
                            start=(h == 0),
                            stop=(h == HC - 1),
                        )
                    if h == HC - 1:
                        for nb in range(2):
                            nc.scalar.copy(
                                out_sb[:, nb * 512 : (nb + 1) * 512], ps_o[nb]
                            )
                        nc.sync.dma_start(out=out.ap(), in_=out_sb)

    nc.compile()
    return nc


_NC_CACHE = None


def _get_nc():
    global _NC_CACHE
    if _NC_CACHE is None:
        _NC_CACHE = build_kernel()
    return _NC_CACHE


def make_in_maps(inputs):
    """Shard full inputs into per-core input maps (head tensor-parallel)."""
    hidden = np.asarray(inputs["hidden_states"], np.float32).reshape(B, D)
    hiddenT = np.zeros((DAUG, B), np.float32)
    hiddenT[:D] = hidden.T
    hiddenT[D] = 1.0  # ones row multiplying the bias row of w_qkv
    # pre-tile to the SBUF layout: hT[p, t, b] = hiddenT[t*128+p, b]
    hT_tiled = np.ascontiguousarray(
        hiddenT.reshape(9, 128, B).transpose(1, 0, 2).reshape(128, 9 * B)
    )
    k_cache = np.asarray(inputs["k_cache"], np.float32)
    v_cache = np.asarray(inputs["v_cache"], np.float32)
    w_attn = np.asarray(inputs["w_attn"], np.float32)
    b_attn = np.asarray(inputs["b_attn"], np.float32)
    w_proj = np.asarray(inputs["w_proj"], np.float32)
    in_maps = []
    for c in range(NCORES):
        hs = slice(c * HC, (c + 1) * HC)           # heads of this core
        cs = slice(c * HC * DH, (c + 1) * HC * DH)  # their q/k/v column block
        w_qkv = np.zeros((DAUG, QKVC), np.float32)
        w_qkv[:D] = np.concatenate(
            [w_attn[:, cs], w_attn[:, D:][:, cs], w_attn[:, 2 * D :][:, cs]], axis=1
        )
        w_qkv[D] = np.concatenate(
            [b_attn[cs], b_attn[D:][cs], b_attn[2 * D :][cs]]
        )
        # pre-tile: wq[p, t, n] = w_qkv[t*128+p, n]
        w_qkv_tiled = np.ascontiguousarray(
            w_qkv.reshape(9, 128, QKVC).transpose(1, 0, 2).reshape(128, 9 * QKVC)
        )
        # pre-tile: wp[p, hh, n] = w_proj[c*128 + hh*64 + p, n]
        wp_tiled = np.ascontiguousarray(
            w_proj[c * HC * DH : (c + 1) * HC * DH]
            .reshape(HC, 64, D)
            .transpose(1, 0, 2)
            .reshape(64, HC * D)
        )
        in_maps.append(
            {
                "hiddenT": hT_tiled,
                "k_cache": np.ascontiguousarray(k_cache[:, hs]),
                "v_cache": np.ascontiguousarray(v_cache[:, hs]),
                "w_qkv": w_qkv_tiled,
                "w_proj_sl": wp_tiled,
            }
        )
    return in_maps


def kernel(**inputs):
    nc = _get_nc()
    in_maps = make_in_maps(inputs)
    res = run_bass_kernel_spmd(nc, in_maps, core_ids=list(range(NCORES)))
    acc = np.zeros((B, D), np.float32)
    for c in range(NCORES):
        acc += np.asarray(res.results[c]["out"])
    acc += np.asarray(inputs["b_proj"], np.float32)[None, :]
    return acc.reshape(B, 1, D)


if __name__ == "__main__":
    build_kernel()
    print("kernel built OK")
